# revision 4
# baseline (speedup 1.0000x reference)
"""Trainium2 Bass kernel for nn_AttentionSimilarity.

Contract: kernel(**inputs) takes the FULL unsharded inputs (numpy) and
returns the FULL [64, 64] similarity matrix, distributing work across 8
NeuronCores internally.

Structure:
  prog1 (projections, sharded by batch): each core projects its 8
    a-batches and 8 b-batches through the three two-layer MLPs,
    emitting qaT/kaT/vaT/qbT/kbT/vbT chunks in [inner, (batch, n)]
    layout. Host gathers the a-side to full tensors.
  prog2 (attention, sharded by p = b-side batch): each core computes
    both attention paths for its 8 p's against all 64 q's, the cosine
    numerators/denominators via selector matmuls on the PE, and the
    per-(p,q) sums over n. Host assembles the [64, 64] output.

Math notes:
  - softmax feeds only cosine similarity, which is scale-invariant in
    the aligned vector, so the softmax max-shift and denominator cancel:
    softmax reduces to exp(scores/8).
  - the x-side cosine norm is folded on the host (vhat = v / max(|v|, eps)).
  - 1/max(|y|, eps) is computed as exp(-0.5 * ln(max(|y|^2, eps^2))),
    keeping ACT in the natural_log_exp table set.
"""

import os
import sys

sys.path.insert(0, "/opt/trn_rl_repo")
os.environ.setdefault("NEURON_RT_RESET_CORES", "1")

import numpy as np

import bass_rust
import concourse.bass as bass
import concourse.mybir as mybir
import concourse.tile as tile
from concourse.bass_utils import run_bass_kernel_spmd

F32 = mybir.dt.float32
F32R = mybir.dt.float32r
AF = mybir.ActivationFunctionType

B = 64          # batches per side
C = 512         # channels
N = 100         # H*W tokens per batch
INNER = 64      # projected dim
CORES = 8
PB = B // CORES  # batches per core (8)
BN = PB * N      # 800: (batch, n) columns per core chunk
EPS = 1e-8

_waitsplit_ctr = [0]


def _split_multi_waits(nc, max_waits=1):
    """This container's walrus build accepts at most ONE sync wait per
    instruction; Tile attaches several. Move extras onto preceding
    same-engine NoOps (engines are in-order, so semantics hold)."""
    n_split = 0
    for f in nc.m.functions:
        for blk in f.blocks:
            insts = list(blk.instructions)
            new_list = []
            changed = False
            for inst in insts:
                si = inst.sync_info
                waits = list(si.on_wait) if (si is not None and si.on_wait) else []
                if len(waits) > max_waits:
                    for w in waits[:-max_waits]:
                        _waitsplit_ctr[0] += 1
                        nop = mybir.InstNoOp(
                            name=f"I-waitsplit-{_waitsplit_ctr[0]}",
                            engine=inst.engine,
                            ins=[],
                            outs=[],
                            sync_info=bass_rust.SyncInfo(on_wait=[w], on_update=[]),
                        )
                        nc.register_instruction(nop, overwrite=True)
                        new_list.append(nop)
                        n_split += 1
                    si.on_wait = waits[-max_waits:]
                    inst.sync_info = si
                    changed = True
                new_list.append(inst)
            if changed:
                blk.instructions = new_list
    return n_split


# ---------------------------------------------------------------- prog1

def build_prog1():
    """Projection program. Per-core inputs:
      fa8, fb8: [PB, C, N] f32 (natural [b, c, n] layout of features)
      wq1/wk1/wv1: [C, C], wq2/wk2/wv2: [C, INNER]
    Outputs: qaT8/kaT8/vaT8/qbT8/kbT8/vbT8: [INNER, BN]  ([i, (b n)])
    """
    nc = bass.Bass("TRN2", target_bir_lowering=False, debug=False,
                   num_devices=CORES)
    fa8 = nc.dram_tensor("fa8", [PB, C, N], F32R, kind="ExternalInput").ap()
    fb8 = nc.dram_tensor("fb8", [PB, C, N], F32R, kind="ExternalInput").ap()
    w1 = {p: nc.dram_tensor(f"w{p}1", [C, C], F32R, kind="ExternalInput").ap()
          for p in "qkv"}
    w2 = {p: nc.dram_tensor(f"w{p}2", [C, INNER], F32R, kind="ExternalInput").ap()
          for p in "qkv"}
    outs = {(s, p): nc.dram_tensor(f"{p}{s}T8", [INNER, BN], F32,
                                   kind="ExternalOutput").ap()
            for s in "ab" for p in "qkv"}

    KT = C // 128  # 4 contraction tiles
    CT = C // 128  # 4 c_out tiles
    CH = [(0, 512), (512, BN)]  # psum-bank-aligned column chunks of BN

    with tile.TileContext(nc) as tc:
        with (
            tc.tile_pool(name="wpool", bufs=1) as wpool,
            tc.tile_pool(name="fpool", bufs=2) as fpool,
            tc.tile_pool(name="hpool", bufs=2) as hpool,
            tc.tile_pool(name="opool", bufs=2) as opool,
            tc.tile_pool(name="psH", bufs=2, space="PSUM") as psHp,
            tc.tile_pool(name="psO", bufs=2, space="PSUM") as psOp,
        ):
            w1sb, w2sb = {}, {}
            for p in "qkv":
                w1sb[p] = wpool.tile([128, KT * C], F32R, tag=f"w1{p}", name=f"w1{p}sb")
                nc.gpsimd.dma_start(
                    w1sb[p][:].rearrange("p (kt o) -> p kt o", kt=KT),
                    w1[p].rearrange("(kt p) o -> p kt o", p=128))
                w2sb[p] = wpool.tile([128, KT * INNER], F32R, tag=f"w2{p}", name=f"w2{p}sb")
                nc.gpsimd.dma_start(
                    w2sb[p][:].rearrange("p (kt i) -> p kt i", kt=KT),
                    w2[p].rearrange("(kt p) i -> p kt i", p=128))

            for s, feat in (("a", fa8), ("b", fb8)):
                fts = []
                for kt in range(KT):
                    ft = fpool.tile([128, BN], F32R, tag=f"f{kt}")
                    nc.gpsimd.dma_start(
                        ft[:].rearrange("c (b n) -> c b n", b=PB),
                        feat[:, 128 * kt:128 * (kt + 1), :].rearrange(
                            "b c n -> c b n"))
                    fts.append(ft)
                for p in "qkv":
                    hts = []
                    for t in range(CT):
                        psH = psHp.tile([128, 1024], F32, tag="psH")
                        for lo, hi in CH:
                            for kt in range(KT):
                                nc.tensor.matmul(
                                    psH[:, lo:hi],
                                    w1sb[p][:, kt * C + 128 * t:
                                            kt * C + 128 * t + 128],
                                    fts[kt][:, lo:hi],
                                    start=(kt == 0), stop=(kt == KT - 1))
                        ht = hpool.tile([128, BN], F32R, tag=f"h{t}")
                        nc.scalar.activation(ht[:], psH[:, 0:BN], AF.Relu)
                        hts.append(ht)
                    psO = psOp.tile([INNER, 1024], F32, tag="psO")
                    for lo, hi in CH:
                        for kt in range(KT):
                            nc.tensor.matmul(
                                psO[:, lo:hi],
                                w2sb[p][:, INNER * kt:INNER * (kt + 1)],
                                hts[kt][:, lo:hi],
                                start=(kt == 0), stop=(kt == KT - 1))
                    ot = opool.tile([INNER, BN], F32, tag="out")
                    nc.scalar.copy(ot[:], psO[:, 0:BN])
                    nc.gpsimd.dma_start(outs[(s, p)][:], ot[:])

    _split_multi_waits(nc)
    return nc


# ---------------------------------------------------------------- prog2

def build_prog2():
    """Attention program, sharded over p (this core's 8 b-batches).

    Inputs (f32r unless noted):
      kaT, qaT      [INNER, B*N]   a-side K^T / Q^T, i on partitions
      qbT, kbT      [INNER, BN]    this core's b-side chunks
      vaL, vaR      [N, 32*128]    per q-pair j: [va[2j] | 0], [0 | va[2j+1]]
      vbL, vbR      [N, 8*128]     per p: [vb[p] | 0], [0 | vb[p]]
      vhat_bT2 f32  [128, BN]      v̂b^T twice (rows 0:64 and 64:128)
      vhat_aT2 f32  [128, B*N//2]  v̂a^T in chunk-pair layout
      master1, master8 [128, 320]  reduce-selector constants
    Outputs (f32):
      out1 [64, PB]   path1 per-(q, p) sums over n of cos1
      out2 [128, 4]   path2 sums; row r: chunk=r//8, p=r%8; q=4*(r//8)+col
    """
    nc = bass.Bass("TRN2", target_bir_lowering=False, debug=False,
                   num_devices=CORES)
    din = {}
    for name, shape, dt in [
        ("kaT", [INNER, B * N], F32R), ("qaT", [INNER, B * N], F32R),
        ("qbT", [INNER, BN], F32R), ("kbT", [INNER, BN], F32R),
        ("vaL", [N, (B // 2) * 128], F32R), ("vaR", [N, (B // 2) * 128], F32R),
        ("vbL", [N, PB * 128], F32R), ("vbR", [N, PB * 128], F32R),
        ("vhat_bT2", [128, BN], F32), ("vhat_aT2", [128, B * N // 2], F32),
        ("master1", [128, 320], F32R), ("master8", [128, 320], F32R),
    ]:
        din[name] = nc.dram_tensor(name, shape, dt, kind="ExternalInput").ap()
    out1 = nc.dram_tensor("out1", [64, PB], F32, kind="ExternalOutput").ap()
    out2 = nc.dram_tensor("out2", [128, 4], F32, kind="ExternalOutput").ap()

    CH1 = [(0, 512), (512, BN)]          # path1 column chunks of (p n)
    NQ2 = (B * N) // 1024                # 6 full 1024-chunks in path2 scores
    E2CH = [(1024 * j, min(1024 * (j + 1), B * N)) for j in range(NQ2 + 1)]

    with tile.TileContext(nc) as tc:
        from contextlib import ExitStack
        with ExitStack() as ctx:
            inp = ctx.enter_context(tc.tile_pool(name="inp", bufs=1))
            sb = {}
            for name in ("kaT", "qaT", "qbT", "kbT", "vaL", "vaR", "vbL",
                         "vbR", "vhat_bT2", "vhat_aT2", "master1", "master8"):
                ap = din[name]
                t = inp.tile(list(ap.shape), ap.dtype, tag=name, name=f"sb_{name}")
                nc.gpsimd.dma_start(t[:], ap[:])
                sb[name] = t

            epool = ctx.enter_context(tc.tile_pool(name="epool", bufs=4))
            mpool = ctx.enter_context(tc.tile_pool(name="mpool", bufs=2))
            fin = ctx.enter_context(tc.tile_pool(name="fin", bufs=1))

            # ---------------- path 1: per q-pair over this core's (p n) ----
            with (
                tc.tile_pool(name="ps_s1", bufs=2, space="PSUM") as ps_s1,
                tc.tile_pool(name="ps_a1", bufs=1, space="PSUM") as ps_a1,
                tc.tile_pool(name="ps_p1", bufs=1, space="PSUM") as ps_p1,
            ):
                P1 = ps_p1.tile([128, 1024], F32, tag="P1")
                for j in range(B // 2):
                    q0, q1 = 2 * j, 2 * j + 1
                    Es = []
                    for q in (q0, q1):
                        S = ps_s1.tile([100, 1024], F32, tag="S1")
                        for lo, hi in CH1:
                            nc.tensor.matmul(
                                S[:, lo:hi],
                                sb["kaT"][:, N * q:N * (q + 1)],
                                sb["qbT"][:, lo:hi], start=True, stop=True)
                        E = epool.tile([100, BN], F32R, tag="E1")
                        nc.scalar.activation(E[:], S[:, 0:BN], AF.Exp,
                                             scale=0.125)
                        Es.append(E)
                    A = ps_a1.tile([128, 1024], F32, tag="A1")
                    for lo, hi in CH1:
                        nc.tensor.matmul(A[:, lo:hi],
                                         sb["vaL"][:, 128 * j:128 * (j + 1)],
                                         Es[0][:, lo:hi],
                                         start=True, stop=False)
                        nc.tensor.matmul(A[:, lo:hi],
                                         sb["vaR"][:, 128 * j:128 * (j + 1)],
                                         Es[1][:, lo:hi],
                                         start=False, stop=True)
                    As = mpool.tile([128, BN], F32R, tag="As1")
                    nc.vector.tensor_copy(As[:], A[:, 0:BN])
                    M = mpool.tile([128, BN], F32R, tag="M1")
                    nc.vector.tensor_mul(M[:], As[:], sb["vhat_bT2"][:])
                    SQ = mpool.tile([128, BN], F32R, tag="SQ1")
                    nc.vector.tensor_mul(SQ[:], As[:], As[:])
                    first, last = (j == 0), (j == B // 2 - 1)
                    for lo, hi in CH1:
                        nc.tensor.matmul(P1[:, lo:hi],
                                         sb["master1"][:, 128 - q0:256 - q0],
                                         M[:, lo:hi],
                                         start=first, stop=False,
                                         skip_group_check=True)
                        nc.tensor.matmul(P1[:, lo:hi],
                                         sb["master1"][:, 64 - q0:192 - q0],
                                         SQ[:, lo:hi],
                                         start=False, stop=last,
                                         skip_group_check=True)

                # epilogue 1: cos1 = dot * exp(-0.5*ln(max(ny2, eps^2)))
                ny = fin.tile([64, BN], F32, tag="ny1")
                nc.vector.tensor_scalar_max(ny[:], P1[64:128, 0:BN], EPS * EPS)
                lg = fin.tile([64, BN], F32, tag="lg1")
                nc.scalar.activation(lg[:], ny[:], AF.Ln)
                rc = fin.tile([64, BN], F32, tag="rc1")
                nc.scalar.activation(rc[:], lg[:], AF.Exp, scale=-0.5)
                cos1 = fin.tile([64, BN], F32, tag="cos1")
                nc.vector.tensor_mul(cos1[:], P1[0:64, 0:BN], rc[:])
                r1 = fin.tile([64, PB], F32, tag="r1")
                nc.vector.tensor_reduce(
                    r1[:], cos1[:].rearrange("q (p n) -> q p n", n=N),
                    mybir.AxisListType.X, mybir.AluOpType.add)
                nc.gpsimd.dma_start(out1[:], r1[:])

            # ---------------- path 2: per p over all (q n) -----------------
            with (
                tc.tile_pool(name="ps_s2", bufs=2, space="PSUM") as ps_s2,
                tc.tile_pool(name="ps_a2", bufs=2, space="PSUM") as ps_a2,
                tc.tile_pool(name="ps_p2", bufs=1, space="PSUM") as ps_p2,
            ):
                P2d = ps_p2.tile([128, 400], F32, tag="P2d")
                P2n = ps_p2.tile([128, 400], F32, tag="P2n")
                for p in range(PB):
                    E2 = epool.tile([100, B * N], F32R, tag="E2", bufs=1)
                    for lo, hi in E2CH:
                        S2 = ps_s2.tile([100, 1024], F32, tag="S2")
                        for l2 in range(lo, hi, 512):
                            h2 = min(l2 + 512, hi)
                            nc.tensor.matmul(
                                S2[:, l2 - lo:h2 - lo],
                                sb["kbT"][:, N * p:N * (p + 1)],
                                sb["qaT"][:, l2:h2], start=True, stop=True)
                        nc.scalar.activation(E2[:, lo:hi], S2[:, 0:hi - lo],
                                             AF.Exp, scale=0.125)
                    for j2 in range(8):  # chunk pairs: cols [800*j2, 800*j2+800)
                        c0 = 800 * j2
                        A2 = ps_a2.tile([128, 400], F32, tag="A2")
                        nc.tensor.matmul(A2[:],
                                         sb["vbL"][:, 128 * p:128 * (p + 1)],
                                         E2[:, c0:c0 + 400],
                                         start=True, stop=False)
                        nc.tensor.matmul(A2[:],
                                         sb["vbR"][:, 128 * p:128 * (p + 1)],
                                         E2[:, c0 + 400:c0 + 800],
                                         start=False, stop=True)
                        As2 = mpool.tile([128, 400], F32R, tag="As2")
                        nc.vector.tensor_copy(As2[:], A2[:])
                        M2 = mpool.tile([128, 400], F32R, tag="M2")
                        nc.vector.tensor_mul(M2[:], As2[:],
                                             sb["vhat_aT2"][:, 400 * j2:
                                                            400 * (j2 + 1)])
                        SQ2 = mpool.tile([128, 400], F32R, tag="SQ2")
                        nc.vector.tensor_mul(SQ2[:], As2[:], As2[:])
                        r0 = 16 * j2 + p
                        first = (p == 0 and j2 == 0)
                        last = (p == PB - 1 and j2 == 7)
                        nc.tensor.matmul(P2d[:],
                                         sb["master8"][:, 128 - r0:256 - r0],
                                         M2[:], start=first, stop=last,
                                         skip_group_check=True)
                        nc.tensor.matmul(P2n[:],
                                         sb["master8"][:, 128 - r0:256 - r0],
                                         SQ2[:], start=first, stop=last,
                                         skip_group_check=True)

                ny2 = fin.tile([128, 400], F32, tag="ny2")
                nc.vector.tensor_scalar_max(ny2[:], P2n[:], EPS * EPS)
                lg2 = fin.tile([128, 400], F32, tag="lg2")
                nc.scalar.activation(lg2[:], ny2[:], AF.Ln)
                rc2 = fin.tile([128, 400], F32, tag="rc2")
                nc.scalar.activation(rc2[:], lg2[:], AF.Exp, scale=-0.5)
                cos2 = fin.tile([128, 400], F32, tag="cos2")
                nc.vector.tensor_mul(cos2[:], P2d[:], rc2[:])
                r2 = fin.tile([128, 4], F32, tag="r2")
                nc.vector.tensor_reduce(
                    r2[:], cos2[:].rearrange("r (g n) -> r g n", n=N),
                    mybir.AxisListType.X, mybir.AluOpType.add)
                nc.gpsimd.dma_start(out2[:], r2[:])

    _split_multi_waits(nc)
    return nc


# ---------------------------------------------------------------- host

_progs = {}


def _get_progs():
    if "p1" not in _progs:
        _progs["p1"] = build_prog1()
        _progs["p2"] = build_prog2()
    return _progs["p1"], _progs["p2"]


def _masters():
    m1 = np.zeros((128, 320), np.float32)
    m1[0:64, 128] = 1.0   # up-plane (rows 0:64 of rhs) -> out row q
    m1[64:128, 129] = 1.0  # down-plane -> out row q+1
    m8 = np.zeros((128, 320), np.float32)
    m8[0:64, 128] = 1.0
    m8[64:128, 136] = 1.0  # down-plane -> out row r0+8
    return m1, m8


def kernel(features_a, features_b, Wq1, Wq2, Wk1, Wk2, Wv1, Wv2):
    nc1, nc2 = _get_progs()
    cc = np.ascontiguousarray

    fa = cc(np.asarray(features_a, np.float32).reshape(B, C, N))
    fb = cc(np.asarray(features_b, np.float32).reshape(B, C, N))
    ws = {"wq1": Wq1, "wk1": Wk1, "wv1": Wv1,
          "wq2": Wq2, "wk2": Wk2, "wv2": Wv2}
    ws = {k: cc(np.asarray(v, np.float32)) for k, v in ws.items()}

    in1 = [dict(fa8=cc(fa[PB * i:PB * (i + 1)]),
                fb8=cc(fb[PB * i:PB * (i + 1)]), **ws)
           for i in range(CORES)]
    res1 = run_bass_kernel_spmd(nc1, in1, core_ids=list(range(CORES)))

    def gather(name):
        return np.concatenate([res1.results[i][name] for i in range(CORES)],
                              axis=1)

    qaT, kaT, vaT = gather("qaT8"), gather("kaT8"), gather("vaT8")
    qbT = [res1.results[i]["qbT8"] for i in range(CORES)]
    kbT = [res1.results[i]["kbT8"] for i in range(CORES)]
    vbT = [res1.results[i]["vbT8"] for i in range(CORES)]

    # a-side derived tensors (shared by all cores)
    va_nm = cc(vaT.T)                       # [B*N, INNER]
    na = np.maximum(np.sqrt((va_nm * va_nm).sum(1)), EPS)
    vhat_aT = vaT / na[None, :]
    vaL = np.zeros((N, (B // 2) * 128), np.float32)
    vaR = np.zeros((N, (B // 2) * 128), np.float32)
    for j in range(B // 2):
        vaL[:, 128 * j:128 * j + 64] = va_nm[N * 2 * j:N * (2 * j + 1)]
        vaR[:, 128 * j + 64:128 * (j + 1)] = va_nm[N * (2 * j + 1):
                                                   N * (2 * j + 2)]
    vhat_aT2 = np.zeros((128, B * N // 2), np.float32)
    for j2 in range(8):
        vhat_aT2[0:64, 400 * j2:400 * (j2 + 1)] = \
            vhat_aT[:, 800 * j2:800 * j2 + 400]
        vhat_aT2[64:128, 400 * j2:400 * (j2 + 1)] = \
            vhat_aT[:, 800 * j2 + 400:800 * (j2 + 1)]
    m1, m8 = _masters()

    in2 = []
    for i in range(CORES):
        vb_nm = cc(vbT[i].T)                # [BN, INNER]
        nb = np.maximum(np.sqrt((vb_nm * vb_nm).sum(1)), EPS)
        vhat_bT = vbT[i] / nb[None, :]
        vbL = np.zeros((N, PB * 128), np.float32)
        vbR = np.zeros((N, PB * 128), np.float32)
        for p in range(PB):
            vbL[:, 128 * p:128 * p + 64] = vb_nm[N * p:N * (p + 1)]
            vbR[:, 128 * p + 64:128 * (p + 1)] = vb_nm[N * p:N * (p + 1)]
        in2.append(dict(
            kaT=cc(kaT), qaT=cc(qaT), qbT=cc(qbT[i]), kbT=cc(kbT[i]),
            vaL=vaL, vaR=vaR, vbL=vbL, vbR=vbR,
            vhat_bT2=cc(np.vstack([vhat_bT, vhat_bT])),
            vhat_aT2=vhat_aT2, master1=m1, master8=m8))
    res2 = run_bass_kernel_spmd(nc2, in2, core_ids=list(range(CORES)))

    sim = np.zeros((B, B), np.float32)
    rr = np.arange(128)
    q_idx = 4 * (rr // 8)[:, None] + np.arange(4)[None, :]
    p_idx = np.broadcast_to((rr % 8)[:, None], (128, 4))
    for i in range(CORES):
        o1 = res2.results[i]["out1"]       # [64(q), PB]
        o2 = res2.results[i]["out2"]       # [128, 4]
        blk = o1.T.copy()                  # [PB, 64] path1 sums
        blk[p_idx, q_idx] += o2
        sim[PB * i:PB * (i + 1)] = blk / N
    return sim


# revision 5
# speedup vs baseline: 1.2373x; 1.2373x over previous
"""Trainium2 Bass kernel for nn_AttentionSimilarity.

Contract: kernel(**inputs) takes the FULL unsharded inputs (numpy) and
returns the FULL [64, 64] similarity matrix, distributing work across 8
NeuronCores internally.

Structure:
  prog1 (projections, sharded by batch): each core projects its 8
    a-batches and 8 b-batches through the three two-layer MLPs,
    emitting qaT/kaT/vaT/qbT/kbT/vbT chunks in [inner, (batch, n)]
    layout. Host gathers the a-side to full tensors.
  prog2 (attention, sharded by p = b-side batch): each core computes
    both attention paths for its 8 p's against all 64 q's, the cosine
    numerators/denominators via selector matmuls on the PE, and the
    per-(p,q) sums over n. Host assembles the [64, 64] output.

Math notes:
  - softmax feeds only cosine similarity, which is scale-invariant in
    the aligned vector, so the softmax max-shift and denominator cancel:
    softmax reduces to exp(scores/8).
  - the x-side cosine norm is folded on the host (vhat = v / max(|v|, eps)).
  - 1/max(|y|, eps) is computed as exp(-0.5 * ln(max(|y|^2, eps^2))),
    keeping ACT in the natural_log_exp table set.
"""

import os
import sys

sys.path.insert(0, "/opt/trn_rl_repo")
os.environ.setdefault("NEURON_RT_RESET_CORES", "1")

import numpy as np

import bass_rust
import concourse.bass as bass
import concourse.mybir as mybir
import concourse.tile as tile
from concourse.bass_utils import run_bass_kernel_spmd

F32 = mybir.dt.float32
F32R = mybir.dt.float32r
BF16 = mybir.dt.bfloat16
AF = mybir.ActivationFunctionType

B = 64          # batches per side
C = 512         # channels
N = 100         # H*W tokens per batch
INNER = 64      # projected dim
CORES = 8
PB = B // CORES  # batches per core (8)
BN = PB * N      # 800: (batch, n) columns per core chunk
EPS = 1e-8

_waitsplit_ctr = [0]


def _split_multi_waits(nc, max_waits=1):
    """This container's walrus build accepts at most ONE sync wait per
    instruction; Tile attaches several. Move extras onto preceding
    same-engine NoOps (engines are in-order, so semantics hold)."""
    n_split = 0
    for f in nc.m.functions:
        for blk in f.blocks:
            insts = list(blk.instructions)
            new_list = []
            changed = False
            for inst in insts:
                si = inst.sync_info
                waits = list(si.on_wait) if (si is not None and si.on_wait) else []
                if len(waits) > max_waits:
                    for w in waits[:-max_waits]:
                        _waitsplit_ctr[0] += 1
                        nop = mybir.InstNoOp(
                            name=f"I-waitsplit-{_waitsplit_ctr[0]}",
                            engine=inst.engine,
                            ins=[],
                            outs=[],
                            sync_info=bass_rust.SyncInfo(on_wait=[w], on_update=[]),
                        )
                        nc.register_instruction(nop, overwrite=True)
                        new_list.append(nop)
                        n_split += 1
                    si.on_wait = waits[-max_waits:]
                    inst.sync_info = si
                    changed = True
                new_list.append(inst)
            if changed:
                blk.instructions = new_list
    return n_split


# ---------------------------------------------------------------- prog1

def build_prog1():
    """Projection program. Per-core inputs:
      fa8, fb8: [PB, C, N] f32 (natural [b, c, n] layout of features)
      wq1/wk1/wv1: [C, C], wq2/wk2/wv2: [C, INNER]
    Outputs: qaT8/kaT8/vaT8/qbT8/kbT8/vbT8: [INNER, BN]  ([i, (b n)])
    """
    nc = bass.Bass("TRN2", target_bir_lowering=False, debug=False,
                   num_devices=CORES)
    fa8 = nc.dram_tensor("fa8", [PB, C, N], F32R, kind="ExternalInput").ap()
    fb8 = nc.dram_tensor("fb8", [PB, C, N], F32R, kind="ExternalInput").ap()
    w1 = {p: nc.dram_tensor(f"w{p}1", [C, C], F32R, kind="ExternalInput").ap()
          for p in "qkv"}
    w2 = {p: nc.dram_tensor(f"w{p}2", [C, INNER], F32R, kind="ExternalInput").ap()
          for p in "qkv"}
    outs = {(s, p): nc.dram_tensor(f"{p}{s}T8", [INNER, BN], F32,
                                   kind="ExternalOutput").ap()
            for s in "ab" for p in "qkv"}

    KT = C // 128  # 4 contraction tiles
    CT = C // 128  # 4 c_out tiles
    CH = [(0, 512), (512, BN)]  # psum-bank-aligned column chunks of BN

    with tile.TileContext(nc) as tc:
        with (
            tc.tile_pool(name="wpool", bufs=1) as wpool,
            tc.tile_pool(name="fpool", bufs=2) as fpool,
            tc.tile_pool(name="hpool", bufs=2) as hpool,
            tc.tile_pool(name="opool", bufs=2) as opool,
            tc.tile_pool(name="psH", bufs=2, space="PSUM") as psHp,
            tc.tile_pool(name="psO", bufs=2, space="PSUM") as psOp,
        ):
            w1sb, w2sb = {}, {}

            def load_w(p):
                w1sb[p] = wpool.tile([128, KT * C], F32R, tag=f"w1{p}",
                                     name=f"w1{p}sb")
                nc.gpsimd.dma_start(
                    w1sb[p][:].rearrange("p (kt o) -> p kt o", kt=KT),
                    w1[p].rearrange("(kt p) o -> p kt o", p=128))
                w2sb[p] = wpool.tile([128, KT * INNER], F32R, tag=f"w2{p}",
                                     name=f"w2{p}sb")
                nc.gpsimd.dma_start(
                    w2sb[p][:].rearrange("p (kt i) -> p kt i", kt=KT),
                    w2[p].rearrange("(kt p) i -> p kt i", p=128))

            load_w("q")
            for s, feat in (("a", fa8), ("b", fb8)):
                fts = []
                for kt in range(KT):
                    ft = fpool.tile([128, BN], F32R, tag=f"f{kt}")
                    nc.gpsimd.dma_start(
                        ft[:].rearrange("c (b n) -> c b n", b=PB),
                        feat[:, 128 * kt:128 * (kt + 1), :].rearrange(
                            "b c n -> c b n"))
                    fts.append(ft)
                if s == "a":
                    load_w("k")
                    load_w("v")
                for p in "qkv":
                    hts = []
                    for t in range(CT):
                        psH = psHp.tile([128, 1024], F32, tag="psH")
                        for lo, hi in CH:
                            for kt in range(KT):
                                nc.tensor.matmul(
                                    psH[:, lo:hi],
                                    w1sb[p][:, kt * C + 128 * t:
                                            kt * C + 128 * t + 128],
                                    fts[kt][:, lo:hi],
                                    start=(kt == 0), stop=(kt == KT - 1))
                        ht = hpool.tile([128, BN], F32R, tag=f"h{t}")
                        nc.scalar.activation(ht[:], psH[:, 0:BN], AF.Relu)
                        hts.append(ht)
                    psO = psOp.tile([INNER, 1024], F32, tag="psO")
                    for lo, hi in CH:
                        for kt in range(KT):
                            nc.tensor.matmul(
                                psO[:, lo:hi],
                                w2sb[p][:, INNER * kt:INNER * (kt + 1)],
                                hts[kt][:, lo:hi],
                                start=(kt == 0), stop=(kt == KT - 1))
                    ot = opool.tile([INNER, BN], F32, tag="out")
                    nc.scalar.copy(ot[:], psO[:, 0:BN])
                    nc.gpsimd.dma_start(outs[(s, p)][:], ot[:])

    _split_multi_waits(nc)
    return nc


# ---------------------------------------------------------------- prog2

def build_prog2():
    """Attention program, sharded over p (this core's 8 b-batches).

    Inputs (f32r unless noted):
      kaT, qaT      [INNER, B*N]   a-side K^T / Q^T, i on partitions
      qbT, kbT      [INNER, BN]    this core's b-side chunks
      vaL, vaR      [N, 32*128]    per q-pair j: [va[2j] | 0], [0 | va[2j+1]]
      vbL, vbR      [N, 8*128]     per p: [vb[p] | 0], [0 | vb[p]]
      vhat_bT2 f32  [128, BN]      v̂b^T twice (rows 0:64 and 64:128)
      vhat_aT2 f32  [128, B*N//2]  v̂a^T in chunk-pair layout
      master1, master8 [128, 320]  reduce-selector constants
    Outputs (f32):
      out1 [64, PB]   path1 per-(q, p) sums over n of cos1
      out2 [128, 4]   path2 sums; row r: chunk=r//8, p=r%8; q=4*(r//8)+col
    """
    nc = bass.Bass("TRN2", target_bir_lowering=False, debug=False,
                   num_devices=CORES)
    din = {}
    for name, shape, dt in [
        ("kaT", [INNER, B * N], F32R), ("qaT", [INNER, B * N], F32R),
        ("qbT", [INNER, BN], F32R), ("kbT", [INNER, BN], F32R),
        ("vaL", [N, (B // 2) * 128], F32R), ("vaR", [N, (B // 2) * 128], F32R),
        ("vbL", [N, PB * 128], F32R), ("vbR", [N, PB * 128], F32R),
        ("vhat_bT2", [128, BN], BF16), ("vhat_aT2", [128, B * N // 2], BF16),
        ("master1", [128, 320], BF16), ("master8", [128, 320], BF16),
    ]:
        din[name] = nc.dram_tensor(name, shape, dt, kind="ExternalInput").ap()
    out1 = nc.dram_tensor("out1", [64, PB], F32, kind="ExternalOutput").ap()
    out2 = nc.dram_tensor("out2", [128, 4], F32, kind="ExternalOutput").ap()

    CH1 = [(0, 512), (512, BN)]          # path1 column chunks of (p n)
    NQ2 = (B * N) // 1024                # 6 full 1024-chunks in path2 scores
    E2CH = [(1024 * j, min(1024 * (j + 1), B * N)) for j in range(NQ2 + 1)]

    with tile.TileContext(nc) as tc:
        from contextlib import ExitStack
        with ExitStack() as ctx:
            inp = ctx.enter_context(tc.tile_pool(name="inp", bufs=1))
            sb = {}

            def load(name, ap=None, cols=None, cname=None):
                ap = din[name] if ap is None else ap
                if cols is not None:
                    ap = ap[:, cols[0]:cols[1]]
                cname = cname or name
                t = inp.tile(list(ap.shape), ap.dtype, tag=cname,
                             name=f"sb_{cname}")
                nc.gpsimd.dma_start(t[:], ap[:])
                sb[cname] = t

            # path1-critical tensors first (chunked so compute starts early)
            load("kaT", cols=(0, 1600), cname="kaT0")
            load("qbT")
            load("vaL", cols=(0, 1024), cname="vaL0")
            load("vaR", cols=(0, 1024), cname="vaR0")
            load("vhat_bT2")
            load("master1")
            load("master8")
            for c in range(1, 4):
                load("kaT", cols=(1600 * c, 1600 * (c + 1)), cname=f"kaT{c}")
                load("vaL", cols=(1024 * c, 1024 * (c + 1)), cname=f"vaL{c}")
                load("vaR", cols=(1024 * c, 1024 * (c + 1)), cname=f"vaR{c}")
            for name in ("qaT", "kbT", "vbL", "vbR", "vhat_aT2"):
                load(name)

            epool = ctx.enter_context(tc.tile_pool(name="epool", bufs=4))
            mpool = ctx.enter_context(tc.tile_pool(name="mpool", bufs=2))
            fin = ctx.enter_context(tc.tile_pool(name="fin", bufs=1))

            # ---------------- path 1: per q-pair over this core's (p n) ----
            with (
                tc.tile_pool(name="ps_s1", bufs=2, space="PSUM") as ps_s1,
                tc.tile_pool(name="ps_a1", bufs=1, space="PSUM") as ps_a1,
                tc.tile_pool(name="ps_p1", bufs=1, space="PSUM") as ps_p1,
            ):
                P1 = ps_p1.tile([128, 1024], F32, tag="P1")
                for j in range(B // 2):
                    q0, q1 = 2 * j, 2 * j + 1
                    Es = []
                    for q in (q0, q1):
                        S = ps_s1.tile([100, 1024], F32, tag="S1")
                        for lo, hi in CH1:
                            nc.tensor.matmul(
                                S[:, lo:hi],
                                sb[f"kaT{q // 16}"][:, N * (q % 16):
                                                    N * (q % 16 + 1)],
                                sb["qbT"][:, lo:hi], start=True, stop=True)
                        E = epool.tile([100, BN], F32R, tag="E1")
                        nc.scalar.activation(E[:], S[:, 0:BN], AF.Exp,
                                             scale=0.125)
                        Es.append(E)
                    A = ps_a1.tile([128, 1024], F32, tag="A1")
                    for lo, hi in CH1:
                        nc.tensor.matmul(A[:, lo:hi],
                                         sb[f"vaL{j // 8}"][:, 128 * (j % 8):
                                                            128 * (j % 8 + 1)],
                                         Es[0][:, lo:hi],
                                         start=True, stop=False)
                        nc.tensor.matmul(A[:, lo:hi],
                                         sb[f"vaR{j // 8}"][:, 128 * (j % 8):
                                                            128 * (j % 8 + 1)],
                                         Es[1][:, lo:hi],
                                         start=False, stop=True)
                    As = mpool.tile([128, BN], BF16, tag="As1")
                    nc.vector.tensor_copy(As[:], A[:, 0:BN])
                    M = mpool.tile([128, BN], BF16, tag="M1")
                    nc.vector.tensor_mul(M[:], As[:], sb["vhat_bT2"][:])
                    SQ = mpool.tile([128, BN], BF16, tag="SQ1")
                    nc.vector.tensor_mul(SQ[:], As[:], As[:])
                    first, last = (j == 0), (j == B // 2 - 1)
                    for lo, hi in CH1:
                        nc.tensor.matmul(P1[:, lo:hi],
                                         sb["master1"][:, 128 - q0:256 - q0],
                                         M[:, lo:hi],
                                         start=first, stop=False,
                                         skip_group_check=True)
                        nc.tensor.matmul(P1[:, lo:hi],
                                         sb["master1"][:, 64 - q0:192 - q0],
                                         SQ[:, lo:hi],
                                         start=False, stop=last,
                                         skip_group_check=True)

                # epilogue 1: cos1 = dot * exp(-0.5*ln(max(ny2, eps^2)))
                ny = fin.tile([64, BN], F32, tag="ny1")
                nc.vector.tensor_scalar_max(ny[:], P1[64:128, 0:BN], EPS * EPS)
                lg = fin.tile([64, BN], F32, tag="lg1")
                nc.scalar.activation(lg[:], ny[:], AF.Ln)
                rc = fin.tile([64, BN], F32, tag="rc1")
                nc.scalar.activation(rc[:], lg[:], AF.Exp, scale=-0.5)
                cos1 = fin.tile([64, BN], F32, tag="cos1")
                nc.vector.tensor_mul(cos1[:], P1[0:64, 0:BN], rc[:])
                r1 = fin.tile([64, PB], F32, tag="r1")
                nc.vector.tensor_reduce(
                    r1[:], cos1[:].rearrange("q (p n) -> q p n", n=N),
                    mybir.AxisListType.X, mybir.AluOpType.add)
                nc.gpsimd.dma_start(out1[:], r1[:])

            # ---------------- path 2: per p over all (q n) -----------------
            with (
                tc.tile_pool(name="ps_s2", bufs=2, space="PSUM") as ps_s2,
                tc.tile_pool(name="ps_a2", bufs=2, space="PSUM") as ps_a2,
                tc.tile_pool(name="ps_p2", bufs=1, space="PSUM") as ps_p2,
            ):
                P2d = ps_p2.tile([128, 400], F32, tag="P2d")
                P2n = ps_p2.tile([128, 400], F32, tag="P2n")
                for p in range(PB):
                    E2 = epool.tile([100, B * N], F32R, tag="E2", bufs=1)
                    for lo, hi in E2CH:
                        S2 = ps_s2.tile([100, 1024], F32, tag="S2")
                        for l2 in range(lo, hi, 512):
                            h2 = min(l2 + 512, hi)
                            nc.tensor.matmul(
                                S2[:, l2 - lo:h2 - lo],
                                sb["kbT"][:, N * p:N * (p + 1)],
                                sb["qaT"][:, l2:h2], start=True, stop=True)
                        nc.scalar.activation(E2[:, lo:hi], S2[:, 0:hi - lo],
                                             AF.Exp, scale=0.125)
                    for j2 in range(8):  # chunk pairs: cols [800*j2, 800*j2+800)
                        c0 = 800 * j2
                        A2 = ps_a2.tile([128, 400], F32, tag="A2")
                        nc.tensor.matmul(A2[:],
                                         sb["vbL"][:, 128 * p:128 * (p + 1)],
                                         E2[:, c0:c0 + 400],
                                         start=True, stop=False)
                        nc.tensor.matmul(A2[:],
                                         sb["vbR"][:, 128 * p:128 * (p + 1)],
                                         E2[:, c0 + 400:c0 + 800],
                                         start=False, stop=True)
                        As2 = mpool.tile([128, 400], BF16, tag="As2")
                        nc.vector.tensor_copy(As2[:], A2[:])
                        M2 = mpool.tile([128, 400], BF16, tag="M2")
                        nc.vector.tensor_mul(M2[:], As2[:],
                                             sb["vhat_aT2"][:, 400 * j2:
                                                            400 * (j2 + 1)])
                        SQ2 = mpool.tile([128, 400], BF16, tag="SQ2")
                        nc.vector.tensor_mul(SQ2[:], As2[:], As2[:])
                        r0 = 16 * j2 + p
                        first = (p == 0 and j2 == 0)
                        last = (p == PB - 1 and j2 == 7)
                        nc.tensor.matmul(P2d[:],
                                         sb["master8"][:, 128 - r0:256 - r0],
                                         M2[:], start=first, stop=last,
                                         skip_group_check=True)
                        nc.tensor.matmul(P2n[:],
                                         sb["master8"][:, 128 - r0:256 - r0],
                                         SQ2[:], start=first, stop=last,
                                         skip_group_check=True)

                ny2 = fin.tile([128, 400], F32, tag="ny2")
                nc.vector.tensor_scalar_max(ny2[:], P2n[:], EPS * EPS)
                lg2 = fin.tile([128, 400], F32, tag="lg2")
                nc.scalar.activation(lg2[:], ny2[:], AF.Ln)
                rc2 = fin.tile([128, 400], F32, tag="rc2")
                nc.scalar.activation(rc2[:], lg2[:], AF.Exp, scale=-0.5)
                cos2 = fin.tile([128, 400], F32, tag="cos2")
                nc.vector.tensor_mul(cos2[:], P2d[:], rc2[:])
                r2 = fin.tile([128, 4], F32, tag="r2")
                nc.vector.tensor_reduce(
                    r2[:], cos2[:].rearrange("r (g n) -> r g n", n=N),
                    mybir.AxisListType.X, mybir.AluOpType.add)
                nc.gpsimd.dma_start(out2[:], r2[:])

    _split_multi_waits(nc)
    return nc


# ---------------------------------------------------------------- host

_progs = {}


def _get_progs():
    if "p1" not in _progs:
        _progs["p1"] = build_prog1()
        _progs["p2"] = build_prog2()
    return _progs["p1"], _progs["p2"]


def _masters():
    import ml_dtypes
    m1 = np.zeros((128, 320), ml_dtypes.bfloat16)
    m1[0:64, 128] = 1.0   # up-plane (rows 0:64 of rhs) -> out row q
    m1[64:128, 129] = 1.0  # down-plane -> out row q+1
    m8 = np.zeros((128, 320), ml_dtypes.bfloat16)
    m8[0:64, 128] = 1.0
    m8[64:128, 136] = 1.0  # down-plane -> out row r0+8
    return m1, m8


def kernel(features_a, features_b, Wq1, Wq2, Wk1, Wk2, Wv1, Wv2):
    nc1, nc2 = _get_progs()
    cc = np.ascontiguousarray

    fa = cc(np.asarray(features_a, np.float32).reshape(B, C, N))
    fb = cc(np.asarray(features_b, np.float32).reshape(B, C, N))
    ws = {"wq1": Wq1, "wk1": Wk1, "wv1": Wv1,
          "wq2": Wq2, "wk2": Wk2, "wv2": Wv2}
    ws = {k: cc(np.asarray(v, np.float32)) for k, v in ws.items()}

    in1 = [dict(fa8=cc(fa[PB * i:PB * (i + 1)]),
                fb8=cc(fb[PB * i:PB * (i + 1)]), **ws)
           for i in range(CORES)]
    res1 = run_bass_kernel_spmd(nc1, in1, core_ids=list(range(CORES)))

    def gather(name):
        return np.concatenate([res1.results[i][name] for i in range(CORES)],
                              axis=1)

    qaT, kaT, vaT = gather("qaT8"), gather("kaT8"), gather("vaT8")
    qbT = [res1.results[i]["qbT8"] for i in range(CORES)]
    kbT = [res1.results[i]["kbT8"] for i in range(CORES)]
    vbT = [res1.results[i]["vbT8"] for i in range(CORES)]

    # a-side derived tensors (shared by all cores)
    va_nm = cc(vaT.T)                       # [B*N, INNER]
    na = np.maximum(np.sqrt((va_nm * va_nm).sum(1)), EPS)
    vhat_aT = vaT / na[None, :]
    vaL = np.zeros((N, (B // 2) * 128), np.float32)
    vaR = np.zeros((N, (B // 2) * 128), np.float32)
    for j in range(B // 2):
        vaL[:, 128 * j:128 * j + 64] = va_nm[N * 2 * j:N * (2 * j + 1)]
        vaR[:, 128 * j + 64:128 * (j + 1)] = va_nm[N * (2 * j + 1):
                                                   N * (2 * j + 2)]
    vhat_aT2 = np.zeros((128, B * N // 2), np.float32)
    for j2 in range(8):
        vhat_aT2[0:64, 400 * j2:400 * (j2 + 1)] = \
            vhat_aT[:, 800 * j2:800 * j2 + 400]
        vhat_aT2[64:128, 400 * j2:400 * (j2 + 1)] = \
            vhat_aT[:, 800 * j2 + 400:800 * (j2 + 1)]
    m1, m8 = _masters()

    in2 = []
    for i in range(CORES):
        vb_nm = cc(vbT[i].T)                # [BN, INNER]
        nb = np.maximum(np.sqrt((vb_nm * vb_nm).sum(1)), EPS)
        vhat_bT = vbT[i] / nb[None, :]
        vbL = np.zeros((N, PB * 128), np.float32)
        vbR = np.zeros((N, PB * 128), np.float32)
        for p in range(PB):
            vbL[:, 128 * p:128 * p + 64] = vb_nm[N * p:N * (p + 1)]
            vbR[:, 128 * p + 64:128 * (p + 1)] = vb_nm[N * p:N * (p + 1)]
        import ml_dtypes
        in2.append(dict(
            kaT=cc(kaT), qaT=cc(qaT), qbT=cc(qbT[i]), kbT=cc(kbT[i]),
            vaL=vaL, vaR=vaR, vbL=vbL, vbR=vbR,
            vhat_bT2=cc(np.vstack([vhat_bT, vhat_bT]).astype(
                ml_dtypes.bfloat16)),
            vhat_aT2=cc(vhat_aT2.astype(ml_dtypes.bfloat16)),
            master1=m1, master8=m8))
    res2 = run_bass_kernel_spmd(nc2, in2, core_ids=list(range(CORES)))

    sim = np.zeros((B, B), np.float32)
    rr = np.arange(128)
    q_idx = 4 * (rr // 8)[:, None] + np.arange(4)[None, :]
    p_idx = np.broadcast_to((rr % 8)[:, None], (128, 4))
    for i in range(CORES):
        o1 = res2.results[i]["out1"]       # [64(q), PB]
        o2 = res2.results[i]["out2"]       # [128, 4]
        blk = o1.T.copy()                  # [PB, 64] path1 sums
        blk[p_idx, q_idx] += o2
        sim[PB * i:PB * (i + 1)] = blk / N
    return sim
